# revision 31
# baseline (speedup 1.0000x reference)
"""MoE (top-2 of 8 experts, D=1024, F=4096, T=8192) on 8 TRN2 NeuronCores.

Strategy: expert-parallel. The router (a [T,1024]@[1024,8] matmul + top-2
softmax, ~0.05% of total FLOPs) runs on host with jax-CPU so expert
selection is bit-identical to the reference. Tokens are dispatched to the
core owning their expert (host-side all-to-all as part of sharding), each
core runs the dense FFN relu(x@w1+b1)@w2+b2 over its padded token batch,
and the host combines contributions weighted by the softmax gates.

Device kernel per core (C tokens): token tiles of 512 (remainder merged
into the last tile so it shares that tile's w1 stream); stage 1 computes
hT[f, tok] accumulating 8 d-chunk matmuls per 128-f-chunk PSUM bank (w1
stationary), fused bias+relu to SBUF; stage 2 computes outT[d, tok]
accumulating 32 f-chunk matmuls into 8 d-chunk PSUM banks (w2
stationary), bias added while draining banks on alternating engines.
Matmul operands are fp16 (fp32 PSUM accumulation, ~4e-4 rel err); fp32
native matmul runs 4-5x slower (LOW_HIGH two-pass at half rate).
Weights stream from HBM per token tile, double-buffered; ~20 MB DMA vs
~109 us of matmul per 512-token tile.
"""

import numpy as np

D_MODEL = 1024
D_FF = 4096
N_EXPERTS = 8
TOP_K = 2
N_CORES = 8
TILE_N = 512
FC = D_FF // 128   # 32 f-chunks
DC = D_MODEL // 128  # 8 d-chunks

TRACE = False
LAST_EXEC_NS = None
LAST_TRACE_PATH = None

# "fp16": matmul operands in fp16, fp32 PSUM accumulation (~4e-4 rel err,
#         ~4x faster than fp32 -- the PE runs fp32 matmuls as two
#         half-rate LOW_HIGH passes).
# "fp32": full fp32 matmuls.
COMPUTE = "fp16"

INTERLEAVE_SEGS = False

_nc_cache = {}


def _build_nc(C, compute):
    import concourse.bacc as bacc
    import concourse.tile as tile
    import concourse.mybir as mybir

    f32 = mybir.dt.float32
    cdt = f32 if compute == "fp32" else mybir.dt.float16
    AFT = mybir.ActivationFunctionType

    nc = bacc.Bacc("TRN2", target_bir_lowering=False, debug=False,
                   num_devices=N_CORES)
    xp = nc.dram_tensor("xp", [D_MODEL, C], cdt, kind="ExternalInput").ap()
    w1p = nc.dram_tensor("w1p", [FC, 128, D_MODEL], cdt,
                         kind="ExternalInput").ap()
    w2p = nc.dram_tensor("w2p", [D_FF, D_MODEL], cdt,
                         kind="ExternalInput").ap()
    b1p = nc.dram_tensor("b1p", [128, FC], f32, kind="ExternalInput").ap()
    b2p = nc.dram_tensor("b2p", [128, DC], f32, kind="ExternalInput").ap()
    yp = nc.dram_tensor("yp", [D_MODEL, C], f32, kind="ExternalOutput").ap()

    xp_r = xp.rearrange("(c p) t -> p c t", p=128)   # [128, 8, C]
    yp_r = yp.rearrange("(c p) t -> p c t", p=128)

    # Token-tile plan: full 512-token tiles; any remainder is merged into
    # the last tile (512+rem tokens) so it shares the last tile's w1
    # stream instead of paying a full weight re-stream for few tokens.
    rem = C % TILE_N
    n_lead = C // TILE_N - (1 if rem else 0)

    with tile.TileContext(nc) as tc:
        with (
            tc.tile_pool(name="const", bufs=1) as constp,
            tc.tile_pool(name="x", bufs=2) as xpool,
            tc.tile_pool(name="h", bufs=1) as hpool,
            tc.tile_pool(name="w1", bufs=6) as w1pool,
            tc.tile_pool(name="w2", bufs=24) as w2pool,
            tc.tile_pool(name="o", bufs=2) as opool,
            tc.tile_pool(name="ps", bufs=8, space="PSUM") as pspool,
        ):
            # constants on the scalar HWDGE queue, off the sync critical path
            b1s = constp.tile([128, FC], f32)
            nc.scalar.dma_start(b1s[:], b1p)
            b2s = constp.tile([128, DC], f32)
            nc.scalar.dma_start(b2s[:], b2p)

            def load_xs(t0, tn):
                xs = xpool.tile([128, DC * tn], cdt, tag="xs",
                                name=f"xs_{t0}")
                # per-chunk DMAs so the first matmul can start as soon as
                # chunk 0 lands (instead of waiting for the full tile)
                for c in range(DC):
                    nc.sync.dma_start(
                        xs[:, c * tn:(c + 1) * tn],
                        xp_r[:, c, t0:t0 + tn],
                    )
                return xs

            def load_w1s(fc):
                w1s = w1pool.tile([128, D_MODEL], cdt, tag="w1s",
                                  name=f"w1s_{fc}")
                nc.sync.dma_start(w1s[:], w1p[fc])
                return w1s

            def stage1(xs, tn, segs, pre=None):
                # segs: list of (token offset in tile, seg len, h tile).
                # Segments are interleaved per d-chunk so a short segment's
                # weight loads hide behind the long segment's matmuls.
                interleave = len(segs) > 1 and INTERLEAVE_SEGS
                for fc in range(FC):
                    w1s = pre[fc] if pre and fc in pre else load_w1s(fc)
                    pss = [pspool.tile([128, sn], f32, tag="ps",
                                       name=f"ps_{fc}_{soff}")
                           for soff, sn, _ in segs]
                    if interleave:
                        for c in range(DC):
                            for ps, (soff, sn, _) in zip(pss, segs):
                                nc.tensor.matmul(
                                    ps[:],
                                    lhsT=w1s[:, c * 128:(c + 1) * 128],
                                    rhs=xs[:, c * tn + soff:
                                           c * tn + soff + sn],
                                    start=(c == 0),
                                    stop=(c == DC - 1),
                                )
                    else:
                        for ps, (soff, sn, _) in zip(pss, segs):
                            for c in range(DC):
                                nc.tensor.matmul(
                                    ps[:],
                                    lhsT=w1s[:, c * 128:(c + 1) * 128],
                                    rhs=xs[:, c * tn + soff:
                                           c * tn + soff + sn],
                                    start=(c == 0),
                                    stop=(c == DC - 1),
                                )
                    for ps, (soff, sn, h) in zip(pss, segs):
                        # alternate relu between Scalar and Vector engines
                        # so consecutive psum banks release in parallel
                        if fc % 2 == 0:
                            nc.scalar.activation(
                                h[:, fc * sn:(fc + 1) * sn], ps[:], AFT.Relu,
                                bias=b1s[:, fc:fc + 1],
                            )
                        else:
                            nc.vector.tensor_scalar(
                                h[:, fc * sn:(fc + 1) * sn], ps[:],
                                b1s[:, fc:fc + 1], 0.0,
                                mybir.AluOpType.add, mybir.AluOpType.max,
                            )

            def stage2(h, sn, t0):
                ps2 = [pspool.tile([128, sn], f32, tag="ps",
                                   name=f"ps2_{t0}_{dc}")
                       for dc in range(DC)]
                for fc in range(FC):
                    w2s = w2pool.tile([128, D_MODEL], cdt, tag="w2s",
                                      name=f"w2s_{fc}")
                    nc.sync.dma_start(w2s[:], w2p[fc * 128:(fc + 1) * 128, :])
                    for dc in range(DC):
                        nc.tensor.matmul(
                            ps2[dc][:],
                            lhsT=w2s[:, dc * 128:(dc + 1) * 128],
                            rhs=h[:, fc * sn:(fc + 1) * sn],
                            start=(fc == 0),
                            stop=(fc == FC - 1),
                        )
                outs = opool.tile([128, DC * sn], f32, tag="o",
                                  name=f"outs_{t0}")
                for dc in range(DC):
                    # alternate engines so the psum banks drain ~2x faster,
                    # and DMA each d-chunk out as soon as its bias is added
                    if dc % 2 == 0:
                        nc.vector.tensor_scalar_add(
                            outs[:, dc * sn:(dc + 1) * sn], ps2[dc][:],
                            b2s[:, dc:dc + 1],
                        )
                    else:
                        nc.scalar.activation(
                            outs[:, dc * sn:(dc + 1) * sn], ps2[dc][:],
                            AFT.Identity, bias=b2s[:, dc:dc + 1],
                        )
                    nc.sync.dma_start(
                        yp_r[:, dc, t0:t0 + sn],
                        outs[:, dc * sn:(dc + 1) * sn],
                    )

            # tiles: (start, len, merged?)
            tiles = [(i * TILE_N, TILE_N, False) for i in range(n_lead)]
            if rem:
                tiles.append((n_lead * TILE_N, TILE_N + rem, True))

            # prologue: x chunk 0 first, then the first w1 strip in
            # per-d-chunk pieces so the first LDWEIGHTS waits on a 32KB
            # transfer instead of the full strip, then the rest
            t0_0, tn_0 = tiles[0][:2]
            xs = xpool.tile([128, DC * tn_0], cdt, tag="xs", name="xs_0")
            nc.sync.dma_start(xs[:, 0:tn_0], xp_r[:, 0, t0_0:t0_0 + tn_0])
            w1s0 = w1pool.tile([128, D_MODEL], cdt, tag="w1s", name="w1s_p0")
            for c in range(DC):
                nc.sync.dma_start(w1s0[:, c * 128:(c + 1) * 128],
                                  w1p[0][:, c * 128:(c + 1) * 128])
            pre0 = {0: w1s0, 1: load_w1s(1)}
            for c in range(1, DC):
                nc.sync.dma_start(xs[:, c * tn_0:(c + 1) * tn_0],
                                  xp_r[:, c, t0_0:t0_0 + tn_0])
            for i, (t0, tn, merged) in enumerate(tiles):
                if merged:
                    ha = hpool.tile([128, FC * TILE_N], cdt, tag="h")
                    hb = hpool.tile([128, FC * rem], cdt, tag="hb")
                    stage1(xs, tn, [(0, TILE_N, ha), (TILE_N, rem, hb)],
                           pre=pre0 if i == 0 else None)
                else:
                    ha = hpool.tile([128, FC * TILE_N], cdt, tag="h")
                    stage1(xs, tn, [(0, TILE_N, ha)],
                           pre=pre0 if i == 0 else None)
                if i + 1 < len(tiles):
                    xs = load_xs(*tiles[i + 1][:2])  # prefetch next x tile
                stage2(ha, TILE_N, t0)
                if merged:
                    stage2(hb, rem, t0 + TILE_N)

    nc.compile()
    return nc


def _ensure_trace_hook():
    """bass_utils' axon trace path needs antenv.axon_hooks; inject it."""
    import sys
    import types
    try:
        import antenv
        if "antenv.axon_hooks" in sys.modules:
            return
        from trn_agent_boot.trn_boot import _ntff_profile_via_ctypes
        mod = types.ModuleType("antenv.axon_hooks")
        hook = [_ntff_profile_via_ctypes("/opt/axon/libaxon_pjrt.so")]
        mod.set_axon_ntff_profile_hook = lambda h: hook.__setitem__(0, h)
        mod.get_axon_ntff_profile_hook = lambda: hook[0]
        sys.modules["antenv.axon_hooks"] = mod
        antenv.axon_hooks = mod
    except Exception:
        pass


def _route(xf, router_w, router_b):
    """Top-2 routing, bit-identical to the reference (jax on CPU)."""
    try:
        import jax
        import jax.numpy as jnp

        cpu = jax.devices("cpu")[0]
        with jax.default_device(cpu):
            logits = (jnp.asarray(xf) @ jnp.asarray(router_w)
                      + jnp.asarray(router_b))
            top_vals, top_idx = jax.lax.top_k(logits, TOP_K)
            wts = jax.nn.softmax(top_vals, axis=-1)
        return np.asarray(top_idx), np.asarray(wts, np.float32)
    except Exception:
        # numpy fallback; ties resolve to the lower index like lax.top_k
        logits = xf @ router_w + router_b
        order = np.argsort(-logits, axis=1, kind="stable")[:, :TOP_K]
        vals = np.take_along_axis(logits, order, axis=1)
        ex = np.exp(vals - vals.max(axis=1, keepdims=True))
        wts = (ex / ex.sum(axis=1, keepdims=True)).astype(np.float32)
        return order, wts


def kernel(x, router_w, router_b, w1, b1, w2, b2):
    global LAST_EXEC_NS, LAST_TRACE_PATH
    from concourse import bass_utils

    x = np.asarray(x, np.float32)
    router_w = np.asarray(router_w, np.float32)
    router_b = np.asarray(router_b, np.float32)
    w1 = np.asarray(w1, np.float32)
    b1 = np.asarray(b1, np.float32)
    w2 = np.asarray(w2, np.float32)
    b2 = np.asarray(b2, np.float32)

    orig_shape = x.shape
    xf = x.reshape(-1, x.shape[-1])
    T = xf.shape[0]

    top_idx, wts = _route(xf, router_w, router_b)

    tok_ids = []
    gates = []
    for e in range(N_EXPERTS):
        mask = top_idx == e                      # [T, K]
        sel = mask.any(axis=1)
        ids = np.nonzero(sel)[0]
        # each token picks distinct experts, so at most one k matches
        gk = np.where(mask[ids, 0], wts[ids, 0], wts[ids, 1]).astype(np.float32)
        tok_ids.append(ids)
        gates.append(gk)

    counts = np.array([len(i) for i in tok_ids])
    C = max(512, int(-(-counts.max() // 128) * 128))

    key = (C, COMPUTE)
    if key not in _nc_cache:
        _nc_cache[key] = _build_nc(C, COMPUTE)
    nc = _nc_cache[key]

    cnp = np.float32 if COMPUTE == "fp32" else np.float16
    in_maps = []
    for e in range(N_EXPERTS):
        ce = counts[e]
        xpad = np.zeros((D_MODEL, C), cnp)
        xpad[:, :ce] = xf[tok_ids[e]].T.astype(cnp)
        w1e = np.ascontiguousarray(
            w1[e].reshape(DC, 128, FC, 128).transpose(2, 1, 0, 3)
            .reshape(FC, 128, D_MODEL).astype(cnp))
        b1e = np.ascontiguousarray(b1[e].reshape(FC, 128).T)
        b2e = np.ascontiguousarray(b2[e].reshape(DC, 128).T)
        in_maps.append({
            "xp": xpad,
            "w1p": w1e,
            "w2p": np.ascontiguousarray(w2[e].astype(cnp)),
            "b1p": b1e,
            "b2p": b2e,
        })

    if TRACE:
        _ensure_trace_hook()
    res = bass_utils.run_bass_kernel_spmd(
        nc, in_maps, core_ids=list(range(N_CORES)), trace=TRACE)
    LAST_EXEC_NS = res.exec_time_ns
    LAST_TRACE_PATH = (res.instructions_and_trace[1]
                       if res.instructions_and_trace else None)

    out = np.zeros((T, D_MODEL), np.float32)
    for e in range(N_EXPERTS):
        ye = np.asarray(res.results[e]["yp"])    # [D, C]
        ce = counts[e]
        out[tok_ids[e]] += gates[e][:, None] * ye.T[:ce]

    return out.reshape(orig_shape)


# revision 32
# speedup vs baseline: 1.0083x; 1.0083x over previous
"""MoE (top-2 of 8 experts, D=1024, F=4096, T=8192) on 8 TRN2 NeuronCores.

Strategy: expert-parallel. The router (a [T,1024]@[1024,8] matmul + top-2
softmax, ~0.05% of total FLOPs) runs on host with jax-CPU so expert
selection is bit-identical to the reference. Tokens are dispatched to the
core owning their expert (host-side all-to-all as part of sharding), each
core runs the dense FFN relu(x@w1+b1)@w2+b2 over its padded token batch,
and the host combines contributions weighted by the softmax gates.

Device kernel per core (C tokens): token tiles of 512 (remainder merged
into the last tile so it shares that tile's w1 stream); stage 1 computes
hT[f, tok] accumulating 8 d-chunk matmuls per 128-f-chunk PSUM bank (w1
stationary), fused bias+relu to SBUF; stage 2 computes outT[d, tok]
accumulating 32 f-chunk matmuls into 8 d-chunk PSUM banks (w2
stationary), bias added while draining banks on alternating engines.
Matmul operands are fp16 (fp32 PSUM accumulation, ~4e-4 rel err); fp32
native matmul runs 4-5x slower (LOW_HIGH two-pass at half rate).
Weights stream from HBM per token tile, double-buffered; ~20 MB DMA vs
~109 us of matmul per 512-token tile.
"""

import numpy as np

D_MODEL = 1024
D_FF = 4096
N_EXPERTS = 8
TOP_K = 2
N_CORES = 8
TILE_N = 512
FC = D_FF // 128   # 32 f-chunks
DC = D_MODEL // 128  # 8 d-chunks

TRACE = False
LAST_EXEC_NS = None
LAST_TRACE_PATH = None

# "fp16": matmul operands in fp16, fp32 PSUM accumulation (~4e-4 rel err,
#         ~4x faster than fp32 -- the PE runs fp32 matmuls as two
#         half-rate LOW_HIGH passes).
# "fp32": full fp32 matmuls.
COMPUTE = "fp16"

INTERLEAVE_SEGS = False

_nc_cache = {}


def _build_nc(C, compute):
    import concourse.bacc as bacc
    import concourse.tile as tile
    import concourse.mybir as mybir

    f32 = mybir.dt.float32
    cdt = f32 if compute == "fp32" else mybir.dt.float16
    AFT = mybir.ActivationFunctionType

    nc = bacc.Bacc("TRN2", target_bir_lowering=False, debug=False,
                   num_devices=N_CORES)
    xp = nc.dram_tensor("xp", [D_MODEL, C], cdt, kind="ExternalInput").ap()
    w1p = nc.dram_tensor("w1p", [FC, 128, D_MODEL], cdt,
                         kind="ExternalInput").ap()
    w2p = nc.dram_tensor("w2p", [D_FF, D_MODEL], cdt,
                         kind="ExternalInput").ap()
    b1p = nc.dram_tensor("b1p", [128, FC], f32, kind="ExternalInput").ap()
    b2p = nc.dram_tensor("b2p", [128, DC], f32, kind="ExternalInput").ap()
    yp = nc.dram_tensor("yp", [D_MODEL, C], f32, kind="ExternalOutput").ap()

    xp_r = xp.rearrange("(c p) t -> p c t", p=128)   # [128, 8, C]
    yp_r = yp.rearrange("(c p) t -> p c t", p=128)

    # Token-tile plan: full 512-token tiles; any remainder is merged into
    # the last tile (512+rem tokens) so it shares the last tile's w1
    # stream instead of paying a full weight re-stream for few tokens.
    rem = C % TILE_N
    n_lead = C // TILE_N - (1 if rem else 0)

    with tile.TileContext(nc) as tc:
        with (
            tc.tile_pool(name="const", bufs=1) as constp,
            tc.tile_pool(name="x", bufs=2) as xpool,
            tc.tile_pool(name="h", bufs=1) as hpool,
            tc.tile_pool(name="w1", bufs=6) as w1pool,
            tc.tile_pool(name="w2", bufs=24) as w2pool,
            tc.tile_pool(name="o", bufs=2) as opool,
            tc.tile_pool(name="ps", bufs=8, space="PSUM") as pspool,
        ):
            # constants on the scalar HWDGE queue, off the sync critical path
            b1s = constp.tile([128, FC], f32)
            nc.scalar.dma_start(b1s[:], b1p)
            b2s = constp.tile([128, DC], f32)
            nc.scalar.dma_start(b2s[:], b2p)

            def load_xs(t0, tn):
                xs = xpool.tile([128, DC * tn], cdt, tag="xs",
                                name=f"xs_{t0}")
                # per-chunk DMAs so the first matmul can start as soon as
                # chunk 0 lands (instead of waiting for the full tile)
                for c in range(DC):
                    nc.sync.dma_start(
                        xs[:, c * tn:(c + 1) * tn],
                        xp_r[:, c, t0:t0 + tn],
                    )
                return xs

            def load_w1s(fc):
                w1s = w1pool.tile([128, D_MODEL], cdt, tag="w1s",
                                  name=f"w1s_{fc}")
                nc.sync.dma_start(w1s[:], w1p[fc])
                return w1s

            def stage1(xs, tn, segs, pre=None):
                # segs: list of (token offset in tile, seg len, h tile).
                # Segments are interleaved per d-chunk so a short segment's
                # weight loads hide behind the long segment's matmuls.
                interleave = len(segs) > 1 and INTERLEAVE_SEGS
                for fc in range(FC):
                    w1s = pre[fc] if pre and fc in pre else load_w1s(fc)
                    pss = [pspool.tile([128, sn], f32, tag="ps",
                                       name=f"ps_{fc}_{soff}")
                           for soff, sn, _ in segs]
                    if interleave:
                        for c in range(DC):
                            for ps, (soff, sn, _) in zip(pss, segs):
                                nc.tensor.matmul(
                                    ps[:],
                                    lhsT=w1s[:, c * 128:(c + 1) * 128],
                                    rhs=xs[:, c * tn + soff:
                                           c * tn + soff + sn],
                                    start=(c == 0),
                                    stop=(c == DC - 1),
                                )
                    else:
                        for ps, (soff, sn, _) in zip(pss, segs):
                            for c in range(DC):
                                nc.tensor.matmul(
                                    ps[:],
                                    lhsT=w1s[:, c * 128:(c + 1) * 128],
                                    rhs=xs[:, c * tn + soff:
                                           c * tn + soff + sn],
                                    start=(c == 0),
                                    stop=(c == DC - 1),
                                )
                    for ps, (soff, sn, h) in zip(pss, segs):
                        # alternate relu between Scalar and Vector engines
                        # so consecutive psum banks release in parallel
                        if fc % 2 == 0:
                            nc.scalar.activation(
                                h[:, fc * sn:(fc + 1) * sn], ps[:], AFT.Relu,
                                bias=b1s[:, fc:fc + 1],
                            )
                        else:
                            nc.vector.tensor_scalar(
                                h[:, fc * sn:(fc + 1) * sn], ps[:],
                                b1s[:, fc:fc + 1], 0.0,
                                mybir.AluOpType.add, mybir.AluOpType.max,
                            )

            def stage2(h, sn, t0):
                ps2 = [pspool.tile([128, sn], f32, tag="ps",
                                   name=f"ps2_{t0}_{dc}")
                       for dc in range(DC)]
                for fc in range(FC):
                    w2s = w2pool.tile([128, D_MODEL], cdt, tag="w2s",
                                      name=f"w2s_{fc}")
                    nc.sync.dma_start(w2s[:], w2p[fc * 128:(fc + 1) * 128, :])
                    for dc in range(DC):
                        nc.tensor.matmul(
                            ps2[dc][:],
                            lhsT=w2s[:, dc * 128:(dc + 1) * 128],
                            rhs=h[:, fc * sn:(fc + 1) * sn],
                            start=(fc == 0),
                            stop=(fc == FC - 1),
                        )
                outs = opool.tile([128, DC * sn], f32, tag="o",
                                  name=f"outs_{t0}")
                for dc in range(DC):
                    # alternate engines so the psum banks drain ~2x faster,
                    # and DMA each d-chunk out as soon as its bias is added
                    if dc % 2 == 0:
                        nc.vector.tensor_scalar_add(
                            outs[:, dc * sn:(dc + 1) * sn], ps2[dc][:],
                            b2s[:, dc:dc + 1],
                        )
                    else:
                        nc.scalar.activation(
                            outs[:, dc * sn:(dc + 1) * sn], ps2[dc][:],
                            AFT.Identity, bias=b2s[:, dc:dc + 1],
                        )
                    nc.sync.dma_start(
                        yp_r[:, dc, t0:t0 + sn],
                        outs[:, dc * sn:(dc + 1) * sn],
                    )

            # tiles: (start, len, merged?)
            tiles = [(i * TILE_N, TILE_N, False) for i in range(n_lead)]
            if rem:
                tiles.append((n_lead * TILE_N, TILE_N + rem, True))

            # hoist the first w1 strips ahead of the x-tile load so the PE
            # can start as soon as x chunk 0 lands
            pre0 = {fc: load_w1s(fc) for fc in range(2)}
            xs = load_xs(*tiles[0][:2])
            for i, (t0, tn, merged) in enumerate(tiles):
                if merged:
                    ha = hpool.tile([128, FC * TILE_N], cdt, tag="h")
                    hb = hpool.tile([128, FC * rem], cdt, tag="hb")
                    stage1(xs, tn, [(0, TILE_N, ha), (TILE_N, rem, hb)],
                           pre=pre0 if i == 0 else None)
                else:
                    ha = hpool.tile([128, FC * TILE_N], cdt, tag="h")
                    stage1(xs, tn, [(0, TILE_N, ha)],
                           pre=pre0 if i == 0 else None)
                if i + 1 < len(tiles):
                    xs = load_xs(*tiles[i + 1][:2])  # prefetch next x tile
                stage2(ha, TILE_N, t0)
                if merged:
                    stage2(hb, rem, t0 + TILE_N)

    nc.compile()
    return nc


def _ensure_trace_hook():
    """bass_utils' axon trace path needs antenv.axon_hooks; inject it."""
    import sys
    import types
    try:
        import antenv
        if "antenv.axon_hooks" in sys.modules:
            return
        from trn_agent_boot.trn_boot import _ntff_profile_via_ctypes
        mod = types.ModuleType("antenv.axon_hooks")
        hook = [_ntff_profile_via_ctypes("/opt/axon/libaxon_pjrt.so")]
        mod.set_axon_ntff_profile_hook = lambda h: hook.__setitem__(0, h)
        mod.get_axon_ntff_profile_hook = lambda: hook[0]
        sys.modules["antenv.axon_hooks"] = mod
        antenv.axon_hooks = mod
    except Exception:
        pass


def _route(xf, router_w, router_b):
    """Top-2 routing, bit-identical to the reference (jax on CPU)."""
    try:
        import jax
        import jax.numpy as jnp

        cpu = jax.devices("cpu")[0]
        with jax.default_device(cpu):
            logits = (jnp.asarray(xf) @ jnp.asarray(router_w)
                      + jnp.asarray(router_b))
            top_vals, top_idx = jax.lax.top_k(logits, TOP_K)
            wts = jax.nn.softmax(top_vals, axis=-1)
        return np.asarray(top_idx), np.asarray(wts, np.float32)
    except Exception:
        # numpy fallback; ties resolve to the lower index like lax.top_k
        logits = xf @ router_w + router_b
        order = np.argsort(-logits, axis=1, kind="stable")[:, :TOP_K]
        vals = np.take_along_axis(logits, order, axis=1)
        ex = np.exp(vals - vals.max(axis=1, keepdims=True))
        wts = (ex / ex.sum(axis=1, keepdims=True)).astype(np.float32)
        return order, wts


def kernel(x, router_w, router_b, w1, b1, w2, b2):
    global LAST_EXEC_NS, LAST_TRACE_PATH
    from concourse import bass_utils

    x = np.asarray(x, np.float32)
    router_w = np.asarray(router_w, np.float32)
    router_b = np.asarray(router_b, np.float32)
    w1 = np.asarray(w1, np.float32)
    b1 = np.asarray(b1, np.float32)
    w2 = np.asarray(w2, np.float32)
    b2 = np.asarray(b2, np.float32)

    orig_shape = x.shape
    xf = x.reshape(-1, x.shape[-1])
    T = xf.shape[0]

    top_idx, wts = _route(xf, router_w, router_b)

    tok_ids = []
    gates = []
    for e in range(N_EXPERTS):
        mask = top_idx == e                      # [T, K]
        sel = mask.any(axis=1)
        ids = np.nonzero(sel)[0]
        # each token picks distinct experts, so at most one k matches
        gk = np.where(mask[ids, 0], wts[ids, 0], wts[ids, 1]).astype(np.float32)
        tok_ids.append(ids)
        gates.append(gk)

    counts = np.array([len(i) for i in tok_ids])
    C = max(512, int(-(-counts.max() // 128) * 128))

    key = (C, COMPUTE)
    if key not in _nc_cache:
        _nc_cache[key] = _build_nc(C, COMPUTE)
    nc = _nc_cache[key]

    cnp = np.float32 if COMPUTE == "fp32" else np.float16
    in_maps = []
    for e in range(N_EXPERTS):
        ce = counts[e]
        xpad = np.zeros((D_MODEL, C), cnp)
        xpad[:, :ce] = xf[tok_ids[e]].T.astype(cnp)
        w1e = np.ascontiguousarray(
            w1[e].reshape(DC, 128, FC, 128).transpose(2, 1, 0, 3)
            .reshape(FC, 128, D_MODEL).astype(cnp))
        b1e = np.ascontiguousarray(b1[e].reshape(FC, 128).T)
        b2e = np.ascontiguousarray(b2[e].reshape(DC, 128).T)
        in_maps.append({
            "xp": xpad,
            "w1p": w1e,
            "w2p": np.ascontiguousarray(w2[e].astype(cnp)),
            "b1p": b1e,
            "b2p": b2e,
        })

    if TRACE:
        _ensure_trace_hook()
    res = bass_utils.run_bass_kernel_spmd(
        nc, in_maps, core_ids=list(range(N_CORES)), trace=TRACE)
    LAST_EXEC_NS = res.exec_time_ns
    LAST_TRACE_PATH = (res.instructions_and_trace[1]
                       if res.instructions_and_trace else None)

    out = np.zeros((T, D_MODEL), np.float32)
    for e in range(N_EXPERTS):
        ye = np.asarray(res.results[e]["yp"])    # [D, C]
        ce = counts[e]
        out[tok_ids[e]] += gates[e][:, None] * ye.T[:ce]

    return out.reshape(orig_shape)
